# revision 70
# baseline (speedup 1.0000x reference)
"""BiWindowMamba layer on 8 Trainium2 cores — v3.

Sharding: core c = (dir, b, half) with dir=c//4, b=(c//2)%2, half=c%2.
Identical SPMD program; backward cores get x flipped in H/W host-side.

v3 structural changes vs v2 (CoreSim cost-model driven, validated against
the walrus BIR verifier — 103118ns -> 91986ns):
  - conv fused into in_proj: 4 tap-shifted PE matmuls per output block with
    host-folded weights diag(cw_j) @ (in_w*ln_g); conv_b (+ ln_b term)
    folded into the silu bias.
  - LN stats via scaled ones-BLOCK matmuls: PE replicates mean/meansq
    across all 128 partitions, so the rstd/mean rows need no broadcast
    DMAs; xn materialized as xpn = (p4 - 4mu) * rstd/4 in bf16.
  - delta path via the sigmoid trick: E = da_1 = sigmoid(-(dd + dt_b)) and
    delta' = ln(E) = -softplus(dd + dt_b); the sign flip is folded into
    negated D and out_w host-side.  Per-state da_s = exp(delta' * s) on
    Act (table sets kept to {sqrt, sigmoid, ln, exp} blocks -> 5 loads).
  - scans stay on DVE (the hardware rejects TensorTensorScanArith on
    Pool); dbu/yt muls mostly on Pool, ~1 mul/state back on DVE, da exps
    on Act - all three within a few % in the scan window.
  - silu via sigmoid + fused scalar_tensor_tensor (ps+cb)*sg from PSUM
    (gpsimd must never touch PSUM - BIR verifier enforces this).
  - x loaded via gpsimd casting DMAs (f32->bf16), quartered, pooling in
    bf16 pipelined behind the transfers; PE p-state warmup matmuls keep
    the tensor engine at 2.4GHz through the head.
  - x_proj emits dt/B/C in one [48,512] PSUM accumulation; dt path
    factored as dt_w @ (xproj_dt @ u) (rank-16).
  - out partials written bf16.
"""

import ml_dtypes
import numpy as np

import concourse.bacc as bacc
import concourse.bass as bass  # noqa: F401
import concourse.mybir as mybir
import concourse.tile as tile

F32 = mybir.dt.float32
BF16 = mybir.dt.bfloat16
AF = mybir.ActivationFunctionType
OP = mybir.AluOpType

C = 256          # model dim
L = 1024         # tokens (32*32 pooled grid)
DF = 512         # full d_inner
DL = 256         # local d_inner shard
NS = 16          # d_state
RK = 16          # dt_rank
KC = 4           # d_conv
NCHIP = 8
BF16NP = ml_dtypes.bfloat16
EPS = 1e-5


# ---- wbig column layout (bf16 [128, WBIG]) ----
def _wx(m, j, k):
    return m * 1024 + j * 256 + k * 128          # [0, 4096)


def _wz(mz, k):
    return 4096 + (mz * 2 + k) * 128             # [4096, 4608)


def _xpw(k):
    return 4608 + k * 48                         # [4608, 4800)


def _dtw(md):
    return 4800 + md * 128                       # rows 0:16


def _ow(md, mc):
    return 5056 + (md * 2 + mc) * 128            # [5056, 5568)


IDC = 5568
WBIG = 5696
# wb arrives in 4 DMA chunks so the x-input DMAs interleave on DMA_ENGINES
WB_CUTS = (0, 2048, 4096, 4608, WBIG)

# ---- wsml layout (f32 [128, WS]) ----
def _av(md, n):
    return md * 16 + n                           # A = -(n+1)


DTB = 32   # +md
DDC = 34   # +md  (D values)
CB2 = 36   # +m   conv+ln_b bias for u silu
CBZ = 40   # +mz  ln_b bias for z silu
WS = 42

# da derivation: 1-based state s; DERIVED[s] = (a, b) with da_s = da_a * da_b.
# da_1 = E = sigmoid(-(dd + dt_b)) comes free from the softplus computation;
# remaining states are Act exp(delta' * s) — Act has the scan-phase slack.
DERIVED = {}


def build_nc():
    nc = bacc.Bacc("TRN2", target_bir_lowering=False, num_swdge_queues=4)

    xin = nc.dram_tensor("xin", [C, 4096], F32, kind="ExternalInput")
    wbig = nc.dram_tensor("wbig", [128, WBIG], BF16, kind="ExternalInput")
    wsml = nc.dram_tensor("wsml", [128, WS], F32, kind="ExternalInput")
    part = nc.dram_tensor("part", [C, L], BF16, kind="ExternalOutput")

    with tile.TileContext(nc) as tc:
        with (
            tc.tile_pool(name="wpool", bufs=1) as wp,
            tc.tile_pool(name="act", bufs=1) as ap,
            tc.tile_pool(name="scan", bufs=4) as sp,
            tc.tile_pool(name="pmm", bufs=4, space="PSUM") as pp,
            tc.tile_pool(name="pyacc", bufs=1, space="PSUM") as py,
            tc.tile_pool(name="dram", bufs=1, space="DRAM") as dp,
        ):
            # ---- input + weight DMAs ----
            # x via gpsimd casting DMAs (f32 -> bf16), quartered so pooling
            # pipelines behind the transfers
            xr = [ap.tile([128, 4096], BF16, tag=f"xraw{g}", name="x_t")
                  for g in range(2)]
            for hh in range(2):      # h-outer: first-half tokens arrive first
                for g in range(2):
                    nc.gpsimd.dma_start(
                        out=xr[g][:, hh * 2048:(hh + 1) * 2048],
                        in_=xin[g * 128:(g + 1) * 128,
                                hh * 2048:(hh + 1) * 2048])
            wb = wp.tile([128, WBIG], BF16, tag="wb", name="wb")
            for ci in range(len(WB_CUTS) - 1):
                nc.sync.dma_start(out=wb[:, WB_CUTS[ci]:WB_CUTS[ci + 1]],
                                  in_=wbig[:, WB_CUTS[ci]:WB_CUTS[ci + 1]])
            ws = wp.tile([128, WS], F32, tag="ws", name="ws")
            nc.scalar.dma_start(out=ws, in_=wsml[:, :])

            # stat weights: scaled ones BLOCKS -> PE replicates the reduced
            # row across all 128 output partitions (broadcast for free)
            ones4c = wp.tile([128, 128], BF16, tag="ones4c", name="ones4c")
            nc.vector.memset(ones4c, 1.0 / (4 * C))
            ones16c = wp.tile([128, 128], BF16, tag="ones16c", name="ones16c")
            nc.vector.memset(ones16c, 1.0 / (16 * C))
            epsc = wp.tile([128, 1], F32, tag="epsc", name="epsc")
            nc.vector.memset(epsc, 16.0 * EPS)
            scr = wp.tile([128, 512], BF16, tag="scr", name="scr")
            nc.vector.memset(scr, 0.0)

            # PE p-state warmup: keep the tensor engine continuously busy
            # through the DMA/pool/LN head so the in_proj taps run at 2.4GHz
            yacc = [[py.tile([128, 512], F32, tag=f"bank{md * 2 + nh}",
                             name="yacc") for nh in range(2)]
                    for md in range(2)]
            for wi in range(30):
                nc.tensor.matmul(yacc[0][0], ones4c[:, 0:128], scr[:, :],
                                 start=True, stop=True)

            # ---- pool: p4 = sum of 2x2 window (bf16), h-outer ----
            p4_tiles = [ap.tile([128, 32, 32], BF16, tag=f"p4_{g}",
                                name="p4_t") for g in range(2)]
            for hh in range(2):
                hs = slice(hh * 16, (hh + 1) * 16)
                for g in range(2):
                    v = xr[g].rearrange("p (h two w tww) -> p h two w tww",
                                        two=2, w=32, tww=2)
                    p4_t = p4_tiles[g]
                    t4 = ap.tile([128, 16, 32], BF16, tag="pooltmp", bufs=2,
                                 name="t4")
                    e1 = nc.vector if g == 0 else nc.gpsimd
                    e2 = nc.gpsimd if g == 0 else nc.vector
                    e1.tensor_add(out=t4, in0=v[:, hs, 0, :, 0],
                                  in1=v[:, hs, 0, :, 1])
                    e2.tensor_add(out=p4_t[:, hs], in0=v[:, hs, 1, :, 0],
                                  in1=v[:, hs, 1, :, 1])
                    e1.tensor_add(out=p4_t[:, hs], in0=p4_t[:, hs], in1=t4)
            p4 = [t.rearrange("p h w -> p (h w)") for t in p4_tiles]

            # ---- LN stats on PE: a = mean, q = meansq/16 (pre-scaled) ----
            xsq = [ap.tile([128, L], BF16, tag=f"xsq{g}", name="xs_t")
                   for g in range(2)]
            for hh in range(2):
                hs = slice(hh * 512, (hh + 1) * 512)
                for g in range(2):
                    eng = nc.vector if g == 0 else nc.gpsimd
                    eng.tensor_mul(out=xsq[g][:, hs], in0=p4[g][:, hs],
                                   in1=p4[g][:, hs])
            # stats in PSUM, already replicated across all 128 partitions:
            # a = mu, q = SSq/(16C)
            a_ps, q_ps = [], []
            for nh in range(2):
                sl = slice(nh * 512, (nh + 1) * 512)
                a_p = pp.tile([128, 512], F32, tag="mm", name="a_p")
                q_p = pp.tile([128, 512], F32, tag="mm", name="q_p")
                for k in range(2):
                    nc.tensor.matmul(a_p, ones4c[:, :], p4[k][:, sl],
                                     start=(k == 0), stop=(k == 1))
                    nc.tensor.matmul(q_p, ones16c[:, :], xsq[k][:, sl],
                                     start=(k == 0), stop=(k == 1))
                a_ps.append(a_p)
                q_ps.append(q_p)

            # more PE warmup filler: spans the LN-chain window so the taps
            # start at full clock
            for wi in range(14):
                nc.tensor.matmul(yacc[0][1], ones4c[:, 0:128], scr[:, :],
                                 start=True, stop=True)

            # ---- LN chain at full width (no broadcast DMAs needed) ----
            # var = q - a^2; rq = 1/sqrt(16*(var+eps)) = rstd/4; mnb = 4*mu
            # (sqrt+reciprocal keeps ln out of the Act-table sequence)
            rqb = ap.tile([128, L], BF16, tag="rqb", name="rqb")
            mnb = ap.tile([128, L], BF16, tag="mnb", name="mnb")
            for nh in range(2):
                sl = slice(nh * 512, (nh + 1) * 512)
                asq = ap.tile([128, 512], BF16, tag="asq", bufs=2, name="asq")
                nc.scalar.activation(out=asq, in_=a_ps[nh], func=AF.Square,
                                     scale=4.0)
                vv = ap.tile([128, 512], BF16, tag="vvc", bufs=2, name="vv")
                nc.vector.tensor_scalar(out=vv, in0=q_ps[nh], scalar1=16.0,
                                        scalar2=0.0, op0=OP.mult, op1=OP.add)
                nc.vector.tensor_sub(out=vv, in0=vv, in1=asq)
                nc.scalar.activation(out=vv, in_=vv, func=AF.Sqrt, bias=epsc)
                with nc.allow_low_precision(reason="bf16 rstd is ample for "
                                            "the 2e-2 tolerance"):
                    nc.vector.reciprocal(out=rqb[:, sl], in_=vv)
            for nh in range(2):
                # after the sqrtss so the Act queue doesn't delay them
                # (gpsimd must not touch PSUM: scale on Act)
                sl = slice(nh * 512, (nh + 1) * 512)
                nc.scalar.activation(out=mnb[:, sl], in_=a_ps[nh],
                                     func=AF.Identity, scale=4.0)

            # ---- xpn = (p4 - mu4) * rq per half (= ln-normalized xn) ----
            xpn = []
            for g in range(2):
                xpn_t = ap.tile([128, 3 + L], BF16, tag=f"xpn{g}",
                                name="xpn_t")
                nc.vector.memset(xpn_t[:, 0:3], 0.0)
                xpn.append(xpn_t)
            for nh in range(2):
                sl = slice(nh * 512, (nh + 1) * 512)
                for g in range(2):
                    tsub = ap.tile([128, 512], BF16, tag="tsub", bufs=4,
                                   name="tsub")
                    eng = nc.vector if g == 0 else nc.gpsimd
                    eng.tensor_sub(out=tsub, in0=p4[g][:, sl],
                                   in1=mnb[:, sl])
                    eng.tensor_mul(out=xpn[g][:, 3 + nh * 512:
                                              3 + (nh + 1) * 512],
                                   in0=tsub, in1=rqb[:, sl])

            # ---- in_proj + conv fused on PE; u = silu(psum + cb2) ----
            # nh-outer: the first-half psums only need first-half xpn
            ut = [ap.tile([128, L], BF16, tag=f"u{m}", name="u_t")
                  for m in range(4)]
            for nh in range(2):
                for m in range(4):
                    u_t = ut[m]
                    ps = pp.tile([128, 512], F32, tag="mm", name="ps")
                    for j in range(4):
                        for k in range(2):
                            nc.tensor.matmul(
                                ps, wb[:, _wx(m, j, k):_wx(m, j, k) + 128],
                                xpn[k][:, j + nh * 512:j + nh * 512 + 512],
                                start=(j == 0 and k == 0),
                                stop=(j == 3 and k == 1))
                    # silu via sigmoid (CoreSim lacks a Silu table):
                    # sg = sigmoid(ps + cb); u = (ps + cb) * sg  (fused STT)
                    sl = slice(nh * 512, (nh + 1) * 512)
                    sg_t = ap.tile([128, 512], BF16, tag="usg", bufs=4,
                                   name="sg_t")
                    nc.scalar.activation(out=sg_t, in_=ps, func=AF.Sigmoid,
                                         bias=ws[:, CB2 + m:CB2 + m + 1])
                    # gpsimd cannot read PSUM -> STT muls stay on DVE
                    nc.vector.scalar_tensor_tensor(
                        out=u_t[:, sl], in0=ps,
                        scalar=ws[:, CB2 + m:CB2 + m + 1], in1=sg_t,
                        op0=OP.add, op1=OP.mult)

            # ---- x_proj: psum [48, 512] per nh = [dt(16) | B(16) | C(16)] ----
            dbc = ap.tile([48, L], BF16, tag="dbc", name="dbc")
            for nh in range(2):
                ps = pp.tile([48, 512], F32, tag="mm", name="ps_xp")
                for k in range(4):
                    nc.tensor.matmul(ps, wb[:, _xpw(k):_xpw(k) + 48],
                                     ut[k][:, nh * 512:(nh + 1) * 512],
                                     start=(k == 0), stop=(k == 3))
                nc.scalar.copy(out=dbc[:, nh * 512:(nh + 1) * 512], in_=ps)
            dbl_d = dp.tile([2 * NS, L], BF16, tag="dbl_d", name="dbl_d")
            nc.sync.dma_start(out=dbl_d, in_=dbc[NS:3 * NS, :])

            # ---- delta path via sigmoid trick ----
            # E = da_1 = exp(-softplus(dd+dtb)) = sigmoid(-(dd+dtb));
            # delta' = ln(E) = -delta.  Signs folded host-side: D and out_w
            # negated, so du' = delta'*u propagates -y through the scan.
            delta = []      # delta' (negated softplus)
            da1 = []        # E tiles (= da for state 1)
            for md in range(2):
                e_t = ap.tile([128, L], BF16, tag=f"da{md}_1", name="e_t")
                for nh in range(2):
                    sl = slice(nh * 512, (nh + 1) * 512)
                    ps = pp.tile([128, 512], F32, tag="mm", name="ps")
                    nc.tensor.matmul(ps, wb[0:16, _dtw(md):_dtw(md) + 128],
                                     dbc[0:16, sl], start=True, stop=True)
                    # wsml DTB holds -dtb
                    nc.scalar.activation(out=e_t[:, sl], in_=ps,
                                         func=AF.Sigmoid, scale=-1.0,
                                         bias=ws[:, DTB + md:DTB + md + 1])
                da1.append(e_t)

            for md in range(2):
                dl_t = ap.tile([128, L], BF16, tag=f"delta{md}", name="dl_t")
                for nh in range(2):
                    sl = slice(nh * 512, (nh + 1) * 512)
                    nc.scalar.activation(out=dl_t[:, sl],
                                         in_=da1[md][:, sl], func=AF.Ln)
                delta.append(dl_t)

            # du' = delta' * u ; yd' = u * (-D) (yacc seed; sign folded)
            du, yd = [], []
            for md in range(2):
                du_t = ap.tile([128, L], BF16, tag=f"du{md}", name="du_t")
                eng = nc.vector if md == 0 else nc.gpsimd
                eng.tensor_mul(out=du_t, in0=delta[md], in1=ut[md])
                du.append(du_t)
                yd_t = ap.tile([128, L], BF16, tag=f"yd{md}", name="yd_t")
                nc.vector.tensor_scalar_mul(
                    out=yd_t, in0=ut[md],
                    scalar1=ws[:, DDC + md:DDC + md + 1])
                yd.append(yd_t)

            # ---- z-part + silu (off the critical path: PE/Act slack) ----
            sz = []
            for mz in range(2):
                sz_t = ap.tile([128, L], BF16, tag=f"sz{mz}", name="sz_t")
                for nh in range(2):
                    ps = pp.tile([128, 512], F32, tag="mm", name="ps")
                    for k in range(2):
                        nc.tensor.matmul(
                            ps, wb[:, _wz(mz, k):_wz(mz, k) + 128],
                            xpn[k][:, 3 + nh * 512:3 + nh * 512 + 512],
                            start=(k == 0), stop=(k == 1))
                    sl = slice(nh * 512, (nh + 1) * 512)
                    sg_t = ap.tile([128, 512], BF16, tag="usg", bufs=4,
                                   name="sg_t")
                    nc.scalar.activation(out=sg_t, in_=ps, func=AF.Sigmoid,
                                         bias=ws[:, CBZ + mz:CBZ + mz + 1])
                    nc.vector.scalar_tensor_tensor(
                        out=sz_t[:, sl], in0=ps,
                        scalar=ws[:, CBZ + mz:CBZ + mz + 1], in1=sg_t,
                        op0=OP.add, op1=OP.mult)
                sz.append(sz_t)

            # ---- selective scan over states (yacc tiles created above) ----
            for md in range(2):
                for nh in range(2):
                    nc.tensor.matmul(yacc[md][nh], wb[:, IDC:IDC + 128],
                                     yd[md][:, nh * 512:(nh + 1) * 512],
                                     start=True, stop=False)

            # da tiles (persist; sources reused for the product chain)
            da = [{1: da1[md]} for md in range(2)]
            for s in range(1, NS + 1):
                n_ = s - 1
                # fused broadcast of B[n], C[n]
                bbc = sp.tile([128, 2, L], BF16, tag="bbc", bufs=5,
                              name="bbc")
                src = dbl_d[n_::NS, :]
                nc.sync.dma_start(out=bbc, in_=src.partition_broadcast(128))
                bb = bbc[:, 0, :]
                cbr = bbc[:, 1, :]
                for md in range(2):
                    if s == 1:
                        da_t = da[md][1]
                    else:
                        # power-of-two states persist (square-chain sources);
                        # the rest rotate through a shared tag
                        if s in (2, 4, 8):
                            da_t = ap.tile([128, L], BF16, tag=f"da{md}_{s}",
                                           name="da_t")
                        else:
                            da_t = sp.tile([128, L], BF16, tag=f"dax{md}",
                                           bufs=3, name="da_t")
                        if s in DERIVED:
                            a, b = DERIVED[s]
                            nc.vector.tensor_mul(out=da_t, in0=da[md][a],
                                                 in1=da[md][b])
                            da[md][s] = da_t
                        else:
                            # wsml AV holds +s (exp(delta'*s) = exp(-delta*s))
                            nc.scalar.activation(
                                out=da_t, in_=delta[md], func=AF.Exp,
                                scale=ws[:, _av(md, n_):_av(md, n_) + 1])
                            if s in (2,):
                                da[md][s] = da_t
                    # scans are DVE-only in hardware; muls go to Pool, with
                    # ~one mul per state back on DVE to balance
                    dbu = sp.tile([128, L], BF16, tag="dbu", bufs=5,
                                  name="dbu")
                    nc.gpsimd.tensor_mul(out=dbu, in0=du[md], in1=bb)
                    h_t = sp.tile([128, L], BF16, tag="h", bufs=5, name="h_t")
                    nc.vector.tensor_tensor_scan(
                        out=h_t, data0=da_t, data1=dbu, initial=0.0,
                        op0=OP.mult, op1=OP.add)
                    yt = sp.tile([128, L], BF16, tag="yt", bufs=5, name="yt")
                    if md == 1 and s % 5 == 0:
                        nc.gpsimd.tensor_mul(out=yt, in0=h_t, in1=cbr)
                    elif md == 1:
                        nc.vector.tensor_mul(out=yt, in0=h_t, in1=cbr)
                    else:
                        nc.gpsimd.tensor_mul(out=yt, in0=h_t, in1=cbr)
                    for nh in range(2):
                        nc.tensor.matmul(
                            yacc[md][nh], wb[:, IDC:IDC + 128],
                            yt[:, nh * 512:(nh + 1) * 512],
                            start=False, stop=(s == NS))

            # ---- y = yacc * silu(z); out-proj; bf16 partial out ----
            yf = []
            for md in range(2):
                yf_t = ap.tile([128, L], BF16, tag=f"yf{md}", name="yf_t")
                for nh in range(2):
                    sl = slice(nh * 512, (nh + 1) * 512)
                    # yacc is PSUM -> DVE only (gpsimd cannot read PSUM)
                    nc.vector.tensor_mul(out=yf_t[:, sl], in0=sz[md][:, sl],
                                         in1=yacc[md][nh])
                yf.append(yf_t)
            for mc in range(2):
                pt = ap.tile([128, L], BF16, tag=f"part{mc}", name="pt")
                for nh in range(2):
                    sl = slice(nh * 512, (nh + 1) * 512)
                    ps = pp.tile([128, 512], F32, tag="mm", name="ps")
                    for md in range(2):
                        nc.tensor.matmul(
                            ps, wb[:, _ow(md, mc):_ow(md, mc) + 128],
                            yf[md][:, sl], start=(md == 0), stop=(md == 1))
                    if nh == 0:
                        nc.scalar.copy(out=pt[:, sl], in_=ps)
                    else:
                        nc.vector.tensor_scalar_mul(out=pt[:, sl], in0=ps,
                                                    scalar1=1.0)
                    eng = nc.sync if mc == 0 else nc.scalar
                    eng.dma_start(out=part[mc * 128:(mc + 1) * 128, sl],
                                  in_=pt[:, sl])
    nc.compile()
    return nc


def make_in_maps(inputs):
    x = np.asarray(inputs["x"], np.float32)
    g = np.asarray(inputs["ln_g"], np.float64)
    be = np.asarray(inputs["ln_b"], np.float64)
    maps = []
    for c in range(NCHIP):
        dr, b, half = c // 4, (c // 2) % 2, c % 2
        p = "f_" if dr == 0 else "b_"
        in_w = np.asarray(inputs[p + "in_w"], np.float64)
        convw = np.asarray(inputs[p + "conv_w"], np.float32)[:, 0, :]  # (512,4)
        convb = np.asarray(inputs[p + "conv_b"], np.float64)
        xpj = np.asarray(inputs[p + "xproj_w"], np.float64)
        dtw = np.asarray(inputs[p + "dt_w"], np.float64)
        dtb = np.asarray(inputs[p + "dt_b"], np.float32)
        alog = np.asarray(inputs[p + "A_log"], np.float64)
        dpar = np.asarray(inputs[p + "D"], np.float32)
        outw = np.asarray(inputs["out_w"], np.float64)

        px = np.concatenate([np.arange(DL) + half * DL,
                             np.arange(DL) + (1 - half) * DL])
        loc = px[:DL]
        xin = x[b] if dr == 0 else x[b, :, ::-1, ::-1]

        # fold ln_g into in_proj cols; ln_b becomes per-row bias ib
        in_wg = in_w * g[None, :]
        ib = in_w @ be                                  # (1024,)
        wx = in_wg[:DF][px]                             # (512, 256)
        wz = in_wg[DF:][loc]                            # (256, 256)
        wsum_x = wx.sum(axis=1)                         # (512,)
        wsum_z = wz.sum(axis=1)
        ib_x = ib[:DF][px]
        ib_z = ib[DF:][loc]
        cw = convw[px]                                  # (512, 4) tap order

        wbig_a = np.zeros((128, WBIG), np.float32)
        # fused conv taps: W_j = diag(cw[:, j]) @ wx, stored transposed
        for m in range(4):
            for j in range(4):
                blk = (wx[m * 128:(m + 1) * 128]
                       * cw[m * 128:(m + 1) * 128, j:j + 1]).T  # (256,128)
                for k in range(2):
                    wbig_a[:, _wx(m, j, k):_wx(m, j, k) + 128] = \
                        blk[k * 128:(k + 1) * 128]
        for mz in range(2):
            blkz = wz[mz * 128:(mz + 1) * 128].T        # (256, 128)
            for k in range(2):
                wbig_a[:, _wz(mz, k):_wz(mz, k) + 128] = \
                    blkz[k * 128:(k + 1) * 128]
        # x_proj: rows = [dt | B | C], contraction over u (px order)
        xpjT = np.ascontiguousarray(xpj[:, px].T)       # (512, 48)
        for k in range(4):
            wbig_a[:, _xpw(k):_xpw(k) + 48] = xpjT[k * 128:(k + 1) * 128]
        # dt_w: (256 loc, 16) -> w [16, 128] per md
        dtwT = dtw[loc].T                               # (16, 256)
        for md in range(2):
            wbig_a[0:16, _dtw(md):_dtw(md) + 128] = \
                dtwT[:, md * 128:(md + 1) * 128]
        # out_proj: (256 out, 256 loc) -> w [128, 128] blocks (NEGATED:
        # the sigmoid-trick delta' = -delta flips the sign of yacc)
        owT = -outw[:, loc]                             # (256, 256)
        for md in range(2):
            for mc in range(2):
                wbig_a[:, _ow(md, mc):_ow(md, mc) + 128] = \
                    owT[mc * 128:(mc + 1) * 128,
                        md * 128:(md + 1) * 128].T
        wbig_a[:, IDC:IDC + 128] = np.eye(128, dtype=np.float32)

        wsml_a = np.zeros((128, WS), np.float32)
        # AV holds -A = +exp(A_log) (= +s for this model's A_log structure);
        # da = exp(delta' * (-A)) since delta' = -delta.
        A = np.exp(alog[loc])                           # (256, 16)
        for md in range(2):
            wsml_a[:, md * 16:(md + 1) * 16] = A[md * 128:(md + 1) * 128]
            wsml_a[:, DTB + md] = -dtb[loc][md * 128:(md + 1) * 128]
            wsml_a[:, DDC + md] = -dpar[loc][md * 128:(md + 1) * 128]
            wsml_a[:, CBZ + md] = ib_z[md * 128:(md + 1) * 128]
        cb2 = convb[px] + ib_x * cw.sum(axis=1)
        for m in range(4):
            wsml_a[:, CB2 + m] = cb2[m * 128:(m + 1) * 128]

        maps.append({
            "xin": np.ascontiguousarray(xin.reshape(C, 4096)),
            "wbig": wbig_a.astype(BF16NP),
            "wsml": wsml_a,
        })
    return maps


def combine(parts, x):
    out = np.empty_like(x)
    for b in range(2):
        acc = np.zeros((C, L), np.float32)
        for c in range(NCHIP):
            dr, bb, _ = c // 4, (c // 2) % 2, c % 2
            if bb != b:
                continue
            pc = np.asarray(parts[c], np.float32)
            if dr == 1:
                pc = pc[:, ::-1]
            acc += pc
        o = acc.reshape(C, 32, 32)
        o = np.repeat(np.repeat(o, 2, axis=1), 2, axis=2)
        out[b] = o + x[b]
    return out


_NC_CACHE = None


def _get_nc():
    global _NC_CACHE
    if _NC_CACHE is None:
        _NC_CACHE = build_nc()
    return _NC_CACHE


def kernel(**inputs):
    from concourse.bass_utils import run_bass_kernel_spmd

    nc = _get_nc()
    in_maps = make_in_maps(inputs)
    res = run_bass_kernel_spmd(nc, in_maps, core_ids=list(range(NCHIP)))
    parts = [r["part"] for r in res.results]
    return combine(parts, np.asarray(inputs["x"], np.float32))


# revision 75
# speedup vs baseline: 1.0105x; 1.0105x over previous
"""BiWindowMamba layer on 8 Trainium2 cores — v3.

Sharding: core c = (dir, b, half) with dir=c//4, b=(c//2)%2, half=c%2.
Identical SPMD program; backward cores get x flipped in H/W host-side.

v3 structural changes vs v2 (CoreSim cost-model driven, validated against
the walrus BIR verifier — 103118ns -> 91986ns):
  - conv fused into in_proj: 4 tap-shifted PE matmuls per output block with
    host-folded weights diag(cw_j) @ (in_w*ln_g); conv_b (+ ln_b term)
    folded into the silu bias.
  - LN stats via scaled ones-BLOCK matmuls: PE replicates mean/meansq
    across all 128 partitions, so the rstd/mean rows need no broadcast
    DMAs; xn materialized as xpn = (p4 - 4mu) * rstd/4 in bf16.
  - delta path via the sigmoid trick: E = da_1 = sigmoid(-(dd + dt_b)) and
    delta' = ln(E) = -softplus(dd + dt_b); the sign flip is folded into
    negated D and out_w host-side.  Per-state da_s = exp(delta' * s) on
    Act (table sets kept to {sqrt, sigmoid, ln, exp} blocks -> 5 loads).
  - scans stay on DVE (the hardware rejects TensorTensorScanArith on
    Pool); dbu/yt muls mostly on Pool, ~1 mul/state back on DVE, da exps
    on Act - all three within a few % in the scan window.
  - silu via sigmoid + fused scalar_tensor_tensor (ps+cb)*sg from PSUM
    (gpsimd must never touch PSUM - BIR verifier enforces this).
  - x loaded via gpsimd casting DMAs (f32->bf16), quartered, pooling in
    bf16 pipelined behind the transfers; PE p-state warmup matmuls keep
    the tensor engine at 2.4GHz through the head.
  - x_proj emits dt/B/C in one [48,512] PSUM accumulation; dt path
    factored as dt_w @ (xproj_dt @ u) (rank-16).
  - out partials written bf16.
"""

import ml_dtypes
import numpy as np

import concourse.bacc as bacc
import concourse.bass as bass  # noqa: F401
import concourse.mybir as mybir
import concourse.tile as tile

F32 = mybir.dt.float32
BF16 = mybir.dt.bfloat16
AF = mybir.ActivationFunctionType
OP = mybir.AluOpType

C = 256          # model dim
L = 1024         # tokens (32*32 pooled grid)
DF = 512         # full d_inner
DL = 256         # local d_inner shard
NS = 16          # d_state
RK = 16          # dt_rank
KC = 4           # d_conv
NCHIP = 8
BF16NP = ml_dtypes.bfloat16
EPS = 1e-5


# ---- wbig column layout (bf16 [128, WBIG]) ----
def _wx(m, j, k):
    return m * 1024 + j * 256 + k * 128          # [0, 4096)


def _wz(mz, k):
    return 4096 + (mz * 2 + k) * 128             # [4096, 4608)


def _xpw(k):
    return 4608 + k * 48                         # [4608, 4800)


def _dtw(md):
    return 4800 + md * 128                       # rows 0:16


def _ow(md, mc):
    return 5056 + (md * 2 + mc) * 128            # [5056, 5568)


IDC = 5568
WBIG = 5696
# wb arrives in 4 DMA chunks so the x-input DMAs interleave on DMA_ENGINES
WB_CUTS = (0, 2048, 4096, 4608, WBIG)

# ---- wsml layout (f32 [128, WS]) ----
def _av(md, n):
    return md * 16 + n                           # A = -(n+1)


DTB = 32   # +md
DDC = 34   # +md  (D values)
CB2 = 36   # +m   conv+ln_b bias for u silu
CBZ = 40   # +mz  ln_b bias for z silu
WS = 42

# da derivation: 1-based state s; DERIVED[s] = (a, b) with da_s = da_a * da_b.
# da_1 = E = sigmoid(-(dd + dt_b)) comes free from the softplus computation;
# remaining states are Act exp(delta' * s) — Act has the scan-phase slack.
DERIVED = {}


def build_nc():
    nc = bacc.Bacc("TRN2", target_bir_lowering=False, num_swdge_queues=4)

    xin = nc.dram_tensor("xin", [C, 4096], F32, kind="ExternalInput")
    wbig = nc.dram_tensor("wbig", [128, WBIG], BF16, kind="ExternalInput")
    wsml = nc.dram_tensor("wsml", [128, WS], F32, kind="ExternalInput")
    part = nc.dram_tensor("part", [C, L], BF16, kind="ExternalOutput")

    with tile.TileContext(nc) as tc:
        with (
            tc.tile_pool(name="wpool", bufs=1) as wp,
            tc.tile_pool(name="act", bufs=1) as ap,
            tc.tile_pool(name="scan", bufs=4) as sp,
            tc.tile_pool(name="pmm", bufs=4, space="PSUM") as pp,
            tc.tile_pool(name="pyacc", bufs=1, space="PSUM") as py,
            tc.tile_pool(name="dram", bufs=1, space="DRAM") as dp,
        ):
            # ---- input + weight DMAs ----
            # x via gpsimd casting DMAs (f32 -> bf16), quartered so pooling
            # pipelines behind the transfers
            xr = [ap.tile([128, 4096], BF16, tag=f"xraw{g}", name="x_t")
                  for g in range(2)]
            for hh in range(2):      # h-outer: first-half tokens arrive first
                for g in range(2):
                    nc.gpsimd.dma_start(
                        out=xr[g][:, hh * 2048:(hh + 1) * 2048],
                        in_=xin[g * 128:(g + 1) * 128,
                                hh * 2048:(hh + 1) * 2048])
            wb = wp.tile([128, WBIG], BF16, tag="wb", name="wb")
            for ci in range(len(WB_CUTS) - 1):
                nc.sync.dma_start(out=wb[:, WB_CUTS[ci]:WB_CUTS[ci + 1]],
                                  in_=wbig[:, WB_CUTS[ci]:WB_CUTS[ci + 1]])
            ws = wp.tile([128, WS], F32, tag="ws", name="ws")
            nc.scalar.dma_start(out=ws, in_=wsml[:, :])

            # stat weights: scaled ones BLOCKS -> PE replicates the reduced
            # row across all 128 output partitions (broadcast for free)
            ones4c = wp.tile([128, 128], BF16, tag="ones4c", name="ones4c")
            nc.vector.memset(ones4c, 1.0 / (4 * C))
            ones16c = wp.tile([128, 128], BF16, tag="ones16c", name="ones16c")
            nc.vector.memset(ones16c, 1.0 / (16 * C))
            epsc = wp.tile([128, 1], F32, tag="epsc", name="epsc")
            nc.vector.memset(epsc, 16.0 * EPS)
            scr = wp.tile([128, 512], BF16, tag="scr", name="scr")
            nc.vector.memset(scr, 0.0)

            # PE p-state warmup: keep the tensor engine continuously busy
            # through the DMA/pool/LN head so the in_proj taps run at 2.4GHz
            yacc = [[py.tile([128, 512], F32, tag=f"bank{md * 2 + nh}",
                             name="yacc") for nh in range(2)]
                    for md in range(2)]
            for wi in range(30):
                nc.tensor.matmul(yacc[0][0], ones4c[:, 0:128], scr[:, :],
                                 start=True, stop=True)

            # ---- pool: p4 = sum of 2x2 window (bf16), h-outer ----
            p4_tiles = [ap.tile([128, 32, 32], BF16, tag=f"p4_{g}",
                                name="p4_t") for g in range(2)]
            for hh in range(2):
                hs = slice(hh * 16, (hh + 1) * 16)
                for g in range(2):
                    v = xr[g].rearrange("p (h two w tww) -> p h two w tww",
                                        two=2, w=32, tww=2)
                    p4_t = p4_tiles[g]
                    t4 = ap.tile([128, 16, 32], BF16, tag="pooltmp", bufs=2,
                                 name="t4")
                    e1 = nc.vector if g == 0 else nc.gpsimd
                    e2 = nc.gpsimd if g == 0 else nc.vector
                    e1.tensor_add(out=t4, in0=v[:, hs, 0, :, 0],
                                  in1=v[:, hs, 0, :, 1])
                    e2.tensor_add(out=p4_t[:, hs], in0=v[:, hs, 1, :, 0],
                                  in1=v[:, hs, 1, :, 1])
                    e1.tensor_add(out=p4_t[:, hs], in0=p4_t[:, hs], in1=t4)
            p4 = [t.rearrange("p h w -> p (h w)") for t in p4_tiles]

            # ---- LN stats on PE: a = mean, q = meansq/16 (pre-scaled) ----
            xsq = [ap.tile([128, L], BF16, tag=f"xsq{g}", name="xs_t")
                   for g in range(2)]
            for hh in range(2):
                hs = slice(hh * 512, (hh + 1) * 512)
                for g in range(2):
                    eng = nc.vector if g == 0 else nc.gpsimd
                    eng.tensor_mul(out=xsq[g][:, hs], in0=p4[g][:, hs],
                                   in1=p4[g][:, hs])
            # stats in PSUM, already replicated across all 128 partitions:
            # a = mu, q = SSq/(16C)
            a_ps, q_ps = [], []
            for nh in range(2):
                sl = slice(nh * 512, (nh + 1) * 512)
                a_p = pp.tile([128, 512], F32, tag="mm", name="a_p")
                q_p = pp.tile([128, 512], F32, tag="mm", name="q_p")
                for k in range(2):
                    nc.tensor.matmul(a_p, ones4c[:, :], p4[k][:, sl],
                                     start=(k == 0), stop=(k == 1))
                    nc.tensor.matmul(q_p, ones16c[:, :], xsq[k][:, sl],
                                     start=(k == 0), stop=(k == 1))
                a_ps.append(a_p)
                q_ps.append(q_p)

            # more PE warmup filler: spans the LN-chain window so the taps
            # start at full clock
            for wi in range(14):
                nc.tensor.matmul(yacc[0][1], ones4c[:, 0:128], scr[:, :],
                                 start=True, stop=True)

            # ---- LN chain at full width (no broadcast DMAs needed) ----
            # var = q - a^2; rq = 1/sqrt(16*(var+eps)) = rstd/4; mnb = 4*mu
            # (sqrt+reciprocal keeps ln out of the Act-table sequence)
            rqb = ap.tile([128, L], BF16, tag="rqb", name="rqb")
            mnb = ap.tile([128, L], BF16, tag="mnb", name="mnb")
            for nh in range(2):
                sl = slice(nh * 512, (nh + 1) * 512)
                asq = ap.tile([128, 512], BF16, tag="asq", bufs=2, name="asq")
                nc.scalar.activation(out=asq, in_=a_ps[nh], func=AF.Square,
                                     scale=4.0)
                vv = ap.tile([128, 512], BF16, tag="vvc", bufs=2, name="vv")
                nc.vector.tensor_scalar(out=vv, in0=q_ps[nh], scalar1=16.0,
                                        scalar2=0.0, op0=OP.mult, op1=OP.add)
                nc.vector.tensor_sub(out=vv, in0=vv, in1=asq)
                nc.scalar.activation(out=vv, in_=vv, func=AF.Sqrt, bias=epsc)
                with nc.allow_low_precision(reason="bf16 rstd is ample for "
                                            "the 2e-2 tolerance"):
                    nc.vector.reciprocal(out=rqb[:, sl], in_=vv)
            for nh in range(2):
                # after the sqrtss so the Act queue doesn't delay them
                # (gpsimd must not touch PSUM: scale on Act)
                sl = slice(nh * 512, (nh + 1) * 512)
                nc.scalar.activation(out=mnb[:, sl], in_=a_ps[nh],
                                     func=AF.Identity, scale=4.0)

            # ---- xpn = (p4 - mu4) * rq per half (= ln-normalized xn) ----
            xpn = []
            for g in range(2):
                xpn_t = ap.tile([128, 3 + L], BF16, tag=f"xpn{g}",
                                name="xpn_t")
                nc.vector.memset(xpn_t[:, 0:3], 0.0)
                xpn.append(xpn_t)
            for nh in range(2):
                sl = slice(nh * 512, (nh + 1) * 512)
                for g in range(2):
                    tsub = ap.tile([128, 512], BF16, tag="tsub", bufs=4,
                                   name="tsub")
                    eng = nc.vector if g == 0 else nc.gpsimd
                    eng.tensor_sub(out=tsub, in0=p4[g][:, sl],
                                   in1=mnb[:, sl])
                    eng.tensor_mul(out=xpn[g][:, 3 + nh * 512:
                                              3 + (nh + 1) * 512],
                                   in0=tsub, in1=rqb[:, sl])

            # ---- in_proj + conv fused on PE; u = silu(psum + cb2) ----
            # nh-outer: the first-half psums only need first-half xpn
            ut = [ap.tile([128, L], BF16, tag=f"u{m}", name="u_t")
                  for m in range(4)]
            for nh in range(2):
                for m in range(4):
                    u_t = ut[m]
                    ps = pp.tile([128, 512], F32, tag="mm", name="ps")
                    for j in range(4):
                        for k in range(2):
                            nc.tensor.matmul(
                                ps, wb[:, _wx(m, j, k):_wx(m, j, k) + 128],
                                xpn[k][:, j + nh * 512:j + nh * 512 + 512],
                                start=(j == 0 and k == 0),
                                stop=(j == 3 and k == 1))
                    # silu via sigmoid (CoreSim lacks a Silu table):
                    # sg = sigmoid(ps + cb); u = (ps + cb) * sg  (fused STT)
                    sl = slice(nh * 512, (nh + 1) * 512)
                    sg_t = ap.tile([128, 512], BF16, tag="usg", bufs=4,
                                   name="sg_t")
                    nc.scalar.activation(out=sg_t, in_=ps, func=AF.Sigmoid,
                                         bias=ws[:, CB2 + m:CB2 + m + 1])
                    # gpsimd cannot read PSUM -> STT muls stay on DVE
                    nc.vector.scalar_tensor_tensor(
                        out=u_t[:, sl], in0=ps,
                        scalar=ws[:, CB2 + m:CB2 + m + 1], in1=sg_t,
                        op0=OP.add, op1=OP.mult)

            # ---- x_proj: psum [48, 512] per nh = [dt(16) | B(16) | C(16)] ----
            dbc = ap.tile([48, L], BF16, tag="dbc", name="dbc")
            for nh in range(2):
                ps = pp.tile([48, 512], F32, tag="mm", name="ps_xp")
                for k in range(4):
                    nc.tensor.matmul(ps, wb[:, _xpw(k):_xpw(k) + 48],
                                     ut[k][:, nh * 512:(nh + 1) * 512],
                                     start=(k == 0), stop=(k == 3))
                nc.scalar.copy(out=dbc[:, nh * 512:(nh + 1) * 512], in_=ps)
            dbl_d = dp.tile([2 * NS, L], BF16, tag="dbl_d", name="dbl_d")
            nc.sync.dma_start(out=dbl_d, in_=dbc[NS:3 * NS, :])

            # ---- delta path via sigmoid trick ----
            # E = da_1 = exp(-softplus(dd+dtb)) = sigmoid(-(dd+dtb));
            # delta' = ln(E) = -delta.  Signs folded host-side: D and out_w
            # negated, so du' = delta'*u propagates -y through the scan.
            delta = []      # delta' (negated softplus)
            da1 = []        # E tiles (= da for state 1)
            for md in range(2):
                e_t = ap.tile([128, L], BF16, tag=f"da{md}_1", name="e_t")
                for nh in range(2):
                    sl = slice(nh * 512, (nh + 1) * 512)
                    ps = pp.tile([128, 512], F32, tag="mm", name="ps")
                    nc.tensor.matmul(ps, wb[0:16, _dtw(md):_dtw(md) + 128],
                                     dbc[0:16, sl], start=True, stop=True)
                    # wsml DTB holds -dtb
                    nc.scalar.activation(out=e_t[:, sl], in_=ps,
                                         func=AF.Sigmoid, scale=-1.0,
                                         bias=ws[:, DTB + md:DTB + md + 1])
                da1.append(e_t)

            for md in range(2):
                dl_t = ap.tile([128, L], BF16, tag=f"delta{md}", name="dl_t")
                for nh in range(2):
                    sl = slice(nh * 512, (nh + 1) * 512)
                    nc.scalar.activation(out=dl_t[:, sl],
                                         in_=da1[md][:, sl], func=AF.Ln)
                delta.append(dl_t)

            # du' = delta' * u ; yd' = u * (-D) (yacc seed; sign folded)
            du, yd = [], []
            for md in range(2):
                du_t = ap.tile([128, L], BF16, tag=f"du{md}", name="du_t")
                eng = nc.vector if md == 0 else nc.gpsimd
                eng.tensor_mul(out=du_t, in0=delta[md], in1=ut[md])
                du.append(du_t)
                yd_t = ap.tile([128, L], BF16, tag=f"yd{md}", name="yd_t")
                nc.vector.tensor_scalar_mul(
                    out=yd_t, in0=ut[md],
                    scalar1=ws[:, DDC + md:DDC + md + 1])
                yd.append(yd_t)

            # ---- z-part + silu (off the critical path: PE/Act slack) ----
            sz = []
            for mz in range(2):
                sz_t = ap.tile([128, L], BF16, tag=f"sz{mz}", name="sz_t")
                for nh in range(2):
                    ps = pp.tile([128, 512], F32, tag="mm", name="ps")
                    for k in range(2):
                        nc.tensor.matmul(
                            ps, wb[:, _wz(mz, k):_wz(mz, k) + 128],
                            xpn[k][:, 3 + nh * 512:3 + nh * 512 + 512],
                            start=(k == 0), stop=(k == 1))
                    sl = slice(nh * 512, (nh + 1) * 512)
                    sg_t = ap.tile([128, 512], BF16, tag="usg", bufs=4,
                                   name="sg_t")
                    nc.scalar.activation(out=sg_t, in_=ps, func=AF.Sigmoid,
                                         bias=ws[:, CBZ + mz:CBZ + mz + 1])
                    nc.vector.scalar_tensor_tensor(
                        out=sz_t[:, sl], in0=ps,
                        scalar=ws[:, CBZ + mz:CBZ + mz + 1], in1=sg_t,
                        op0=OP.add, op1=OP.mult)
                sz.append(sz_t)

            # ---- selective scan over states (yacc tiles created above) ----
            for md in range(2):
                for nh in range(2):
                    nc.tensor.matmul(yacc[md][nh], wb[:, IDC:IDC + 128],
                                     yd[md][:, nh * 512:(nh + 1) * 512],
                                     start=True, stop=False)

            # da tiles (persist; sources reused for the product chain)
            da = [{1: da1[md]} for md in range(2)]
            for s in range(1, NS + 1):
                n_ = s - 1
                # fused broadcast of B[n], C[n]
                bbc = sp.tile([128, 2, L], BF16, tag="bbc", bufs=5,
                              name="bbc")
                src = dbl_d[n_::NS, :]
                nc.sync.dma_start(out=bbc, in_=src.partition_broadcast(128))
                bb = bbc[:, 0, :]
                cbr = bbc[:, 1, :]
                for md in range(2):
                    if s == 1:
                        da_t = da[md][1]
                    else:
                        # power-of-two states persist (square-chain sources);
                        # the rest rotate through a shared tag
                        if s in (2, 4, 8):
                            da_t = ap.tile([128, L], BF16, tag=f"da{md}_{s}",
                                           name="da_t")
                        else:
                            da_t = sp.tile([128, L], BF16, tag=f"dax{md}",
                                           bufs=3, name="da_t")
                        if s in DERIVED:
                            a, b = DERIVED[s]
                            nc.vector.tensor_mul(out=da_t, in0=da[md][a],
                                                 in1=da[md][b])
                            da[md][s] = da_t
                        else:
                            # wsml AV holds +s (exp(delta'*s) = exp(-delta*s))
                            nc.scalar.activation(
                                out=da_t, in_=delta[md], func=AF.Exp,
                                scale=ws[:, _av(md, n_):_av(md, n_) + 1])
                            if s in (2,):
                                da[md][s] = da_t
                    # scans are DVE-only in hardware; muls go to Pool, with
                    # ~one mul per state back on DVE to balance
                    dbu = sp.tile([128, L], BF16, tag="dbu", bufs=5,
                                  name="dbu")
                    nc.gpsimd.tensor_mul(out=dbu, in0=du[md], in1=bb)
                    h_t = sp.tile([128, L], BF16, tag="h", bufs=5, name="h_t")
                    nc.vector.tensor_tensor_scan(
                        out=h_t, data0=da_t, data1=dbu, initial=0.0,
                        op0=OP.mult, op1=OP.add)
                    yt = sp.tile([128, L], BF16, tag="yt", bufs=5, name="yt")
                    if md == 1 and s % 3 == 0:
                        nc.gpsimd.tensor_mul(out=yt, in0=h_t, in1=cbr)
                    elif md == 1:
                        nc.vector.tensor_mul(out=yt, in0=h_t, in1=cbr)
                    else:
                        nc.gpsimd.tensor_mul(out=yt, in0=h_t, in1=cbr)
                    for nh in range(2):
                        nc.tensor.matmul(
                            yacc[md][nh], wb[:, IDC:IDC + 128],
                            yt[:, nh * 512:(nh + 1) * 512],
                            start=False, stop=(s == NS))

            # ---- y = yacc * silu(z); out-proj; bf16 partial out ----
            yf = []
            for md in range(2):
                yf_t = ap.tile([128, L], BF16, tag=f"yf{md}", name="yf_t")
                for nh in range(2):
                    sl = slice(nh * 512, (nh + 1) * 512)
                    # yacc is PSUM -> DVE only (gpsimd cannot read PSUM)
                    nc.vector.tensor_mul(out=yf_t[:, sl], in0=sz[md][:, sl],
                                         in1=yacc[md][nh])
                yf.append(yf_t)
            for mc in range(2):
                pt = ap.tile([128, L], BF16, tag=f"part{mc}", name="pt")
                for nh in range(2):
                    sl = slice(nh * 512, (nh + 1) * 512)
                    ps = pp.tile([128, 512], F32, tag="mm", name="ps")
                    for md in range(2):
                        nc.tensor.matmul(
                            ps, wb[:, _ow(md, mc):_ow(md, mc) + 128],
                            yf[md][:, sl], start=(md == 0), stop=(md == 1))
                    if nh == 0:
                        nc.scalar.copy(out=pt[:, sl], in_=ps)
                    else:
                        nc.vector.tensor_scalar_mul(out=pt[:, sl], in0=ps,
                                                    scalar1=1.0)
                    eng = nc.sync if mc == 0 else nc.scalar
                    eng.dma_start(out=part[mc * 128:(mc + 1) * 128, sl],
                                  in_=pt[:, sl])
    nc.compile()
    return nc


def make_in_maps(inputs):
    x = np.asarray(inputs["x"], np.float32)
    g = np.asarray(inputs["ln_g"], np.float64)
    be = np.asarray(inputs["ln_b"], np.float64)
    maps = []
    for c in range(NCHIP):
        dr, b, half = c // 4, (c // 2) % 2, c % 2
        p = "f_" if dr == 0 else "b_"
        in_w = np.asarray(inputs[p + "in_w"], np.float64)
        convw = np.asarray(inputs[p + "conv_w"], np.float32)[:, 0, :]  # (512,4)
        convb = np.asarray(inputs[p + "conv_b"], np.float64)
        xpj = np.asarray(inputs[p + "xproj_w"], np.float64)
        dtw = np.asarray(inputs[p + "dt_w"], np.float64)
        dtb = np.asarray(inputs[p + "dt_b"], np.float32)
        alog = np.asarray(inputs[p + "A_log"], np.float64)
        dpar = np.asarray(inputs[p + "D"], np.float32)
        outw = np.asarray(inputs["out_w"], np.float64)

        px = np.concatenate([np.arange(DL) + half * DL,
                             np.arange(DL) + (1 - half) * DL])
        loc = px[:DL]
        xin = x[b] if dr == 0 else x[b, :, ::-1, ::-1]

        # fold ln_g into in_proj cols; ln_b becomes per-row bias ib
        in_wg = in_w * g[None, :]
        ib = in_w @ be                                  # (1024,)
        wx = in_wg[:DF][px]                             # (512, 256)
        wz = in_wg[DF:][loc]                            # (256, 256)
        wsum_x = wx.sum(axis=1)                         # (512,)
        wsum_z = wz.sum(axis=1)
        ib_x = ib[:DF][px]
        ib_z = ib[DF:][loc]
        cw = convw[px]                                  # (512, 4) tap order

        wbig_a = np.zeros((128, WBIG), np.float32)
        # fused conv taps: W_j = diag(cw[:, j]) @ wx, stored transposed
        for m in range(4):
            for j in range(4):
                blk = (wx[m * 128:(m + 1) * 128]
                       * cw[m * 128:(m + 1) * 128, j:j + 1]).T  # (256,128)
                for k in range(2):
                    wbig_a[:, _wx(m, j, k):_wx(m, j, k) + 128] = \
                        blk[k * 128:(k + 1) * 128]
        for mz in range(2):
            blkz = wz[mz * 128:(mz + 1) * 128].T        # (256, 128)
            for k in range(2):
                wbig_a[:, _wz(mz, k):_wz(mz, k) + 128] = \
                    blkz[k * 128:(k + 1) * 128]
        # x_proj: rows = [dt | B | C], contraction over u (px order)
        xpjT = np.ascontiguousarray(xpj[:, px].T)       # (512, 48)
        for k in range(4):
            wbig_a[:, _xpw(k):_xpw(k) + 48] = xpjT[k * 128:(k + 1) * 128]
        # dt_w: (256 loc, 16) -> w [16, 128] per md
        dtwT = dtw[loc].T                               # (16, 256)
        for md in range(2):
            wbig_a[0:16, _dtw(md):_dtw(md) + 128] = \
                dtwT[:, md * 128:(md + 1) * 128]
        # out_proj: (256 out, 256 loc) -> w [128, 128] blocks (NEGATED:
        # the sigmoid-trick delta' = -delta flips the sign of yacc)
        owT = -outw[:, loc]                             # (256, 256)
        for md in range(2):
            for mc in range(2):
                wbig_a[:, _ow(md, mc):_ow(md, mc) + 128] = \
                    owT[mc * 128:(mc + 1) * 128,
                        md * 128:(md + 1) * 128].T
        wbig_a[:, IDC:IDC + 128] = np.eye(128, dtype=np.float32)

        wsml_a = np.zeros((128, WS), np.float32)
        # AV holds -A = +exp(A_log) (= +s for this model's A_log structure);
        # da = exp(delta' * (-A)) since delta' = -delta.
        A = np.exp(alog[loc])                           # (256, 16)
        for md in range(2):
            wsml_a[:, md * 16:(md + 1) * 16] = A[md * 128:(md + 1) * 128]
            wsml_a[:, DTB + md] = -dtb[loc][md * 128:(md + 1) * 128]
            wsml_a[:, DDC + md] = -dpar[loc][md * 128:(md + 1) * 128]
            wsml_a[:, CBZ + md] = ib_z[md * 128:(md + 1) * 128]
        cb2 = convb[px] + ib_x * cw.sum(axis=1)
        for m in range(4):
            wsml_a[:, CB2 + m] = cb2[m * 128:(m + 1) * 128]

        maps.append({
            "xin": np.ascontiguousarray(xin.reshape(C, 4096)),
            "wbig": wbig_a.astype(BF16NP),
            "wsml": wsml_a,
        })
    return maps


def combine(parts, x):
    out = np.empty_like(x)
    for b in range(2):
        acc = np.zeros((C, L), np.float32)
        for c in range(NCHIP):
            dr, bb, _ = c // 4, (c // 2) % 2, c % 2
            if bb != b:
                continue
            pc = np.asarray(parts[c], np.float32)
            if dr == 1:
                pc = pc[:, ::-1]
            acc += pc
        o = acc.reshape(C, 32, 32)
        o = np.repeat(np.repeat(o, 2, axis=1), 2, axis=2)
        out[b] = o + x[b]
    return out


_NC_CACHE = None


def _get_nc():
    global _NC_CACHE
    if _NC_CACHE is None:
        _NC_CACHE = build_nc()
    return _NC_CACHE


def kernel(**inputs):
    from concourse.bass_utils import run_bass_kernel_spmd

    nc = _get_nc()
    in_maps = make_in_maps(inputs)
    res = run_bass_kernel_spmd(nc, in_maps, core_ids=list(range(NCHIP)))
    parts = [r["part"] for r in res.results]
    return combine(parts, np.asarray(inputs["x"], np.float32))
